# revision 8
# baseline (speedup 1.0000x reference)
"""Trainium2 Bass kernel for nn_Attention_89670327206161.

Dense transformer attention block, B=8 S=4096 D=1024 H=16 (dh=64), fp32.
The reference contracts attention scores over the *sequence* axis:
    scores_h = K_h^T Q_h / sqrt(dh)   -> (dh, dh) per head
    P_h      = softmax(scores_h, axis=-1)
    out_h    = V_h @ P_h              -> (S, dh)
    out      = concat_h(out_h) @ Wo^T

Because the score matrices are tiny, the whole computation collapses
algebraically:
    scores_h = (Wk^T)_h^T G (Wq^T)_h     with  G = x^T x  (1024x1024 Gram)
    Mv       = [ (Wv^T)_1 P_1 | ... | (Wv^T)_16 P_16 ]   (1024x1024)
    W_eff    = Mv @ Wo^T                                  (1024x1024)
    out      = x @ W_eff
so per core (one batch element) the FLOPs are: G (8.6GF) + G@WqT (2.1GF)
+ scores (0.5GF) + Mv (0.27GF) + W_eff (2.1GF) + final (8.6GF), about
half of the naive Q/K/V/O projection cost.

Precision split: the score path (x, Wq, Wk, G) stays fp32r because the
raw logits reach |1136| (exp-sensitive); the V/O path (P, Wv, Wo, W_eff,
final matmul, output) is linear and runs in bf16 (~0.4% rel err).

Sharding: pure data parallelism over batch; one batch element per core,
no collectives.
"""

import numpy as np

HEADS = 16
B, S, D = 8, 4096, 1024
DH = D // HEADS          # 64
NPAIR = HEADS // 2       # 8 head pairs -> 128-wide blocks
P = 128
NKC = D // P             # 8 feature chunks of 128
NSC = S // P             # 32 seq chunks of 128
NPP = 4                  # pair-pairs (4 heads each) for 256-wide tmp
N_CORES = 8
# G accumulation round sizes: first rounds small so the PE ramps with the
# x DMA stream instead of stalling on all 8 tiles of a full round
ROUNDS = (2, 6, 8, 8, 8)

# Upper-triangle panels of G: (row_chunk, col_start, width, flush_offset).
# Only [c0+off, c0+W) is accumulated into G_sb; the rest of the matmul
# output is a recompute of a neighbouring panel (cheaper than an N<256
# fp32r matmul). Widths stay >=256 so fp32r streams at full rate.
G_PANELS = (
    (0, 0, 512, 0), (0, 512, 512, 0),
    (1, 128, 512, 0), (1, 640, 384, 0),
    (2, 256, 512, 0), (2, 768, 256, 0),
    (3, 384, 512, 0), (3, 768, 256, 128),
    (4, 512, 512, 0),
    (5, 640, 384, 0),
    (6, 768, 256, 0),
    (7, 768, 256, 128),
)

_PROGRAM = None


def _ts(i, n):
    return slice(i * n, (i + 1) * n)


def _build_program():
    import concourse.bacc as bacc
    import concourse.mybir as mybir
    import concourse.tile as tile

    f32 = mybir.dt.float32
    f32r = mybir.dt.float32r
    bf16 = mybir.dt.bfloat16
    EXP = mybir.ActivationFunctionType.Exp
    X = mybir.AxisListType.X

    nc = bacc.Bacc(trn_type="TRN2", debug=False, num_devices=N_CORES)

    x_sm = nc.dram_tensor("x_sm", [S, D], f32r, kind="ExternalInput")
    wqT = nc.dram_tensor("wqT", [D, D], f32r, kind="ExternalInput")
    wkT = nc.dram_tensor("wkT", [D, D], f32r, kind="ExternalInput")
    wv = nc.dram_tensor("wv", [D, D], bf16, kind="ExternalInput")
    woT = nc.dram_tensor("woT", [D, D], bf16, kind="ExternalInput")
    xTb = nc.dram_tensor("xTb", [D, S], bf16, kind="ExternalInput")
    out = nc.dram_tensor("out", [S, D], bf16, kind="ExternalOutput")

    x_r = x_sm.ap().rearrange("(sc p) d -> p sc d", p=P)    # (128, 32, 1024)
    wqTr = wqT.ap().rearrange("(c p) o -> p c o", p=P)      # (128, 8, 1024)
    wkTr = wkT.ap().rearrange("(c p) o -> p c o", p=P)
    wvr = wv.ap().rearrange("(r p) i -> p r i", p=P)        # (128, 8, 1024)
    woTr = woT.ap().rearrange("(c p) o -> p c o", p=P)
    xTr = xTb.ap().rearrange("(c p) s -> p c s", p=P)       # (128, 8, 4096)

    with tile.TileContext(nc) as tc:
      with (
          tc.tile_pool(name="persist", bufs=1) as persist,
          tc.tile_pool(name="const", bufs=1) as const_pool,
      ):
        # ---------------- phase G: G = x^T x ----------------
        G_sb = persist.tile([P, NKC, D], f32r, tag="G")
        p_all = persist.tile([P, NPAIR, P], bf16, tag="pall")

        zero_sb = const_pool.tile([P, 512], f32r)
        nc.vector.memset(zero_sb[:].bitcast(f32), 0.0)
        nc.vector.memset(p_all[:], 0.0)

        with (
            tc.tile_pool(name="wqk", bufs=1) as wqk_pool,
            tc.tile_pool(name="vo", bufs=1) as vo_pool,
        ):
            wq_sb = wqk_pool.tile([P, NKC, D], f32r, tag="wq")
            wk_sb = wqk_pool.tile([P, NKC, D], f32r, tag="wk")
            wv_sb = vo_pool.tile([P, NKC, D], bf16, tag="wv")
            woT_sb = vo_pool.tile([P, NKC, D], bf16, tag="wo")
            mvT_sb = vo_pool.tile([P, NKC, D], bf16, tag="mvT")

            with (
                tc.tile_pool(name="xg", bufs=12) as xg_pool,
                tc.tile_pool(name="g_ps", bufs=6, space="PSUM") as g_ps_pool,
                tc.tile_pool(name="tr_ps", bufs=2, space="PSUM") as tr_ps_pool,
            ):
                # All input DMAs on the single sync HWDGE queue, in priority
                # order: x stream first (feeds phase G immediately), then the
                # weights in the order later phases consume them. One FIFO
                # queue means the critical x stream never shares HBM
                # bandwidth with the weight prefetch.
                xg_tiles = []
                for s in range(NSC):
                    xg = xg_pool.tile([P, D], f32r, tag="xg")
                    nc.sync.dma_start(xg[:], x_r[:, s, :])
                    xg_tiles.append(xg)
                nc.sync.dma_start(wq_sb[:], wqTr)
                nc.sync.dma_start(wk_sb[:], wkTr)
                nc.sync.dma_start(wv_sb[:], wvr)
                nc.sync.dma_start(woT_sb[:], woTr)

                # HAM warm-up: spin the PE while the first x tiles arrive
                warm_ps = g_ps_pool.tile([P, 512], f32, tag="gps")
                for _ in range(14):
                    nc.tensor.matmul(
                        warm_ps[:], zero_sb[:, 0:P], zero_sb[:],
                        start=True, stop=False, skip_group_check=True,
                    )

                s_base = 0
                first_round = True
                for rnd in ROUNDS:
                    chunk = xg_tiles[s_base:s_base + rnd]
                    for rc, c0, w, off in G_PANELS:
                        ps = g_ps_pool.tile([P, 512], f32, tag="gps")
                        for s in range(rnd):
                            nc.tensor.matmul(
                                ps[:, 0:w], chunk[s][:, _ts(rc, P)],
                                chunk[s][:, c0:c0 + w],
                                start=(s == 0), stop=(s == rnd - 1),
                            )
                        dst = G_sb[:, rc, c0 + off:c0 + w]
                        src = ps[:, off:w]
                        if first_round:
                            if (rc + c0 // 512) % 2 == 0:
                                nc.scalar.copy(dst, src)
                            else:
                                nc.vector.tensor_copy(dst, src)
                        else:
                            nc.vector.tensor_add(dst, dst, src)
                    s_base += rnd
                    first_round = False

                # mirror the lower triangle: G[jc, :, ic-cols] for ic < jc is
                # the PE-transpose of the stored upper block G[ic, :, jc-cols]
                from concourse import masks
                id32 = const_pool.tile([P, P], f32)
                masks.make_identity(nc, id32[:])
                idr = const_pool.tile([P, P], f32r)
                nc.vector.tensor_copy(idr[:], id32[:])
                n_tr = 0
                for jc in range(1, NKC):
                    for ic in range(jc):
                        tps = tr_ps_pool.tile([P, P], f32r, tag="trps")
                        nc.tensor.transpose(
                            tps[:], G_sb[:, ic, _ts(jc, P)], idr[:])
                        dst = G_sb[:, jc, _ts(ic, P)]
                        if n_tr % 2 == 0:
                            nc.scalar.copy(dst, tps[:])
                        else:
                            nc.vector.tensor_copy(dst, tps[:])
                        n_tr += 1

            # ---------------- tmp = G @ WqT, scores, softmax ----------------
            scores_done = []
            with (
                tc.tile_pool(name="tmp", bufs=2) as tmp_pool,
                tc.tile_pool(name="qk_ps", bufs=4, space="PSUM") as qk_ps_pool,
                tc.tile_pool(name="sc_ps", bufs=1, space="PSUM") as sc_ps_pool,
                tc.tile_pool(name="smx", bufs=4) as smx_pool,
            ):
                scores_ps = sc_ps_pool.tile([P, NPAIR * 256], f32)
                for pp in range(NPP):
                    tmp_sb = tmp_pool.tile([P, NKC, 256], f32r, tag="tmp")
                    # descending ic: the first matmuls only read upper-G
                    # blocks (ic >= jc), so the PE doesn't stall on the
                    # mirror copies still landing on DVE/ACT
                    for ic in range(NKC - 1, -1, -1):
                        ps = qk_ps_pool.tile([P, 256], f32, tag="qkps")
                        for jc in range(NKC):
                            nc.tensor.matmul(
                                ps[:], G_sb[:, jc, _ts(ic, P)],
                                wq_sb[:, jc, _ts(pp, 256)],
                                start=(jc == 0), stop=(jc == NKC - 1),
                            )
                        if ic % 2 == 0:
                            nc.scalar.copy(tmp_sb[:, ic, :], ps[:])
                        else:
                            nc.vector.tensor_copy(tmp_sb[:, ic, :], ps[:])
                    for j2 in range(2):
                        pr = 2 * pp + j2
                        for ic in range(NKC):
                            nc.tensor.matmul(
                                scores_ps[:, _ts(pr, 256)],
                                wk_sb[:, ic, _ts(pr, P)],
                                tmp_sb[:, ic, :],
                                start=(ic == 0), stop=(ic == NKC - 1),
                            )
                    # softmax for this pair-pair (overlaps next pp's matmuls)
                    for j2 in range(2):
                        pr = 2 * pp + j2
                        base = pr * 256 + (pr % 2) * P
                        for hf in range(2):
                            rows = slice(64 * hf, 64 * hf + 64)
                            cols = slice(base + 64 * hf, base + 64 * hf + 64)
                            pcols = slice(64 * hf, 64 * hf + 64)
                            mx = smx_pool.tile([P, 1], f32, tag="mx")
                            nmx = smx_pool.tile([P, 1], f32, tag="nmx")
                            nc.vector.reduce_max(
                                mx[rows, 0:1], scores_ps[rows, cols],
                                axis=X, negate=True,
                            )
                            nc.vector.tensor_scalar_mul(
                                nmx[rows, 0:1], mx[rows, 0:1], 0.125)
                            p_tmp = smx_pool.tile([P, 64], f32, tag="ptmp")
                            nc.scalar.activation(
                                p_tmp[rows, :], scores_ps[rows, cols], EXP,
                                bias=nmx[rows, 0:1], scale=0.125,
                            )
                            den = smx_pool.tile([P, 1], f32, tag="den")
                            rec = smx_pool.tile([P, 1], f32, tag="rec")
                            nc.vector.reduce_sum(
                                den[rows, 0:1], p_tmp[rows, :], axis=X)
                            nc.vector.reciprocal(rec[rows, 0:1], den[rows, 0:1])
                            nc.vector.tensor_scalar_mul(
                                p_all[rows, pr, pcols], p_tmp[rows, :],
                                rec[rows, 0:1],
                            )

            # ---------------- MvT = blockdiag(P)^T applied to Wv ------------
            with tc.tile_pool(name="mv_ps", bufs=4, space="PSUM") as mv_ps_pool:
                for pr in range(NPAIR):
                    for ih in range(2):
                        ps = mv_ps_pool.tile([P, 512], f32, tag="mvps")
                        nc.tensor.matmul(
                            ps[:], p_all[:, pr, :], wv_sb[:, pr, _ts(ih, 512)],
                            start=True, stop=True,
                        )
                        if ih == 0:
                            nc.scalar.copy(mvT_sb[:, pr, _ts(ih, 512)], ps[:])
                        else:
                            nc.vector.tensor_copy(
                                mvT_sb[:, pr, _ts(ih, 512)], ps[:])

        # ---------------- W_eff = Mv @ WoT ; out = x @ W_eff ----------------
        with (
            tc.tile_pool(name="weff", bufs=1) as weff_pool,
            tc.tile_pool(name="xt", bufs=1) as xt_pool,
            tc.tile_pool(name="ob", bufs=3) as ob_pool,
            tc.tile_pool(name="b_ps", bufs=4, space="PSUM") as b_ps_pool,
        ):
            weff = [weff_pool.tile([P, NKC, 512], bf16, tag=f"weff{oh}", name=f"weff{oh}")
                    for oh in range(2)]
            xt_sec = [xt_pool.tile([P, NKC, 512], bf16, tag=f"xt{sec}", name=f"xt{sec}")
                      for sec in range(8)]
            for sec in range(8):
                nc.sync.dma_start(xt_sec[sec][:], xTr[:, :, _ts(sec, 512)])

            def emit_weff(oh):
                for rc in range(NKC):
                    ps = b_ps_pool.tile([P, 512], f32, tag="bps")
                    for jc in range(NKC):
                        nc.tensor.matmul(
                            ps[:], mvT_sb[:, jc, _ts(rc, P)],
                            woT_sb[:, jc, _ts(oh, 512)],
                            start=(jc == 0), stop=(jc == NKC - 1),
                        )
                    if rc % 2 == 0:
                        nc.scalar.copy(weff[oh][:, rc, :], ps[:])
                    else:
                        nc.vector.tensor_copy(weff[oh][:, rc, :], ps[:])

            emit_weff(0)
            emit_weff(1)
            for oh in range(2):
                for st in range(NSC):
                    ps = b_ps_pool.tile([P, 512], f32, tag="bps")
                    sec, off = st // 4, (st % 4) * P
                    for ic in range(NKC):
                        nc.tensor.matmul(
                            ps[:], xt_sec[sec][:, ic, off:off + P],
                            weff[oh][:, ic, :],
                            start=(ic == 0), stop=(ic == NKC - 1),
                        )
                    o_sb = ob_pool.tile([P, 512], bf16, tag="ob")
                    if st % 2 == 0:
                        nc.scalar.copy(o_sb[:], ps[:])
                    else:
                        nc.vector.tensor_copy(o_sb[:], ps[:])
                    nc.sync.dma_start(
                        out.ap()[_ts(st, P), _ts(oh, 512)], o_sb[:])

    nc.compile()
    return nc


def _get_program():
    global _PROGRAM
    if _PROGRAM is None:
        _PROGRAM = _build_program()
    return _PROGRAM


def kernel(x, Wq, Wk, Wv, Wo):
    import ml_dtypes
    from concourse import bass_utils

    nc = _get_program()
    bf = ml_dtypes.bfloat16

    x32 = np.asarray(x, np.float32)
    wqT = np.ascontiguousarray(np.asarray(Wq, np.float32).T)
    wkT = np.ascontiguousarray(np.asarray(Wk, np.float32).T)
    wv_b = np.ascontiguousarray(np.asarray(Wv, np.float32)).astype(bf)
    woT_b = np.ascontiguousarray(np.asarray(Wo, np.float32).T).astype(bf)

    in_maps = []
    for b in range(N_CORES):
        xb = np.ascontiguousarray(x32[b])
        xTb = np.ascontiguousarray(x32[b].T).astype(bf)
        in_maps.append({
            "x_sm": xb, "wqT": wqT, "wkT": wkT,
            "wv": wv_b, "woT": woT_b, "xTb": xTb,
        })
    res = bass_utils.run_bass_kernel_spmd(nc, in_maps, core_ids=list(range(N_CORES)))
    return np.stack(
        [np.asarray(res.results[b]["out"], np.float32) for b in range(N_CORES)],
        axis=0,
    )
